# revision 45
# baseline (speedup 1.0000x reference)
"""Trainium2 Bass kernel for nn_BranchDiversity.

Computes, for x: [n=8, B=128, C=512, H=7, W=7]:
  xm   = mean(x, axis=C)                       -> [n, B, H, W]
  sq   = sum((xm_i - xm_j)^2, axis=(H,W))      -> [n, n, B]
  snm  = mean(exp(-GAMMA*sq), axis=B) * (1-I)  -> [n, n]
  out  = (sum(snm), -det(snm), -logdet(snm) if det>0 else nan)

Sharding: data-parallel over B across 8 NeuronCores (16 batch each).
Per core the kernel does the memory-bound channel-sum reduction
([128 partitions = (b_local, n) b-major, 25088 free = (C, H*W)]; tapered
chunks so the last exposed reduce is short), then the pairwise tail fully
on-chip: stream_shuffle rotates branches within each batch's 8-partition
group (r=1..4; sq is symmetric so the host mirrors the other half), and
sq(i, (i+r)%8, b) comes from per-partition sub/sq/reduce with no
cross-partition DMA. Emits exp(-GAMMA*sq)[(b_local, i), r-1] = [128, 4].
Host assembles snm (mean over all 128 batches, diag 0) and does the tiny
8x8 det/logdet.
"""

import numpy as np
from contextlib import ExitStack

import concourse.bass as bass
import concourse.tile as tile
from concourse import bacc, mybir
from concourse.bass_utils import run_bass_kernel_spmd

# Problem constants (hardcoded per contract; kernel.py must be self-contained)
N_CORES = 8
N = 8          # branches
B = 128        # batch
C = 512        # channels
HW = 49        # H*W = 7*7
B_SH = B // N_CORES        # 16 batch per core
P = N * B_SH               # 128 partitions = (b_local, n), b-major
CF = C * HW                # 25088 free elems per partition
GAMMA = 10.0
# exp(-GAMMA * sq_ref) with sq_ref = sq_rawsum / C^2 (xm kept as raw channel
# sums on device; 1/C^2 folded into the activation scale, exact: C^2 = 2^18)
EXP_SCALE = -GAMMA / (C * C)

# Channel chunking: loads are striped across BOTH HWDGE rings (nc.sync and
# nc.scalar) so the two DMA queues stream in parallel; the vector engine's
# reduce throughput (1 f32 elem/cycle @0.96GHz ~= 26 us for all 512ch) then
# becomes the pipeline bottleneck. Small chunks first so DVE starts early,
# then large chunks to amortize per-op fixed cost. Chosen by searching the
# cost-model pipeline (see dev notes).
# (channels, ring, mode) in arrival order. Ring 0 = SP/nc.sync (ready ~0.2us),
# ring 1 = ACT/nc.scalar (ready ~1.5us after its activation-table load),
# ring 2 = SWDGE/nc.gpsimd (third parallel stream; its chunks reach gpsimd
# folds with no DMA->compute sem hop, same engine). Modes: 'f1' gp folds the
# chunk's channel-halves once, DVE reduces the half; 'f2' gp folds twice, DVE
# reduces the quarter; 'g' gp folds a pow2 chunk all the way to one channel
# (no DVE work); 'd' direct DVE reduce. Mix balances DVE (~0.96GHz) and
# GPSIMD (~1.2GHz) loads against the 3-way-striped DMA span. Found by
# searching the cost-model pipeline; DVE_ORDER is the model's ready order
# (engines execute their stream in program order).
PLAN = [
    (96, 2, "f"),                # SWDGE third stream; fold needs no DMA sem
    (64, 0, "f"), (64, 1, "f"),
    (64, 0, "f2"), (64, 1, "f"),
    (48, 0, "f2"), (44, 1, "f2"),
    (32, 0, "g"), (8, 0, "d"),
    (24, 1, "f"), (4, 1, "d"),
]
assert sum(cc for cc, _, _ in PLAN) == C
# per-engine emission orders (engines run their streams in program order):
GP_ORDER = [1, 0, 2, 3, 4, 5, 6, 7, 9]       # folds by expected data-ready
DVE_EARLY = [1, 0, 2, 3, 4, 5, 6, 8]          # reduces before early combine
DVE_LATE = [10, 9]                            # tail reduces after it

F32 = mybir.dt.float32


def build_nc():
    """Build the per-core Bass program (SPMD: same program, different data)."""
    nc = bacc.Bacc("TRN2", target_bir_lowering=False)
    x = nc.dram_tensor("x", [P, CF], F32, kind="ExternalInput")
    e_out = nc.dram_tensor("e_out", [P, N // 2], F32, kind="ExternalOutput")

    with tile.TileContext(nc) as tc, ExitStack() as ctx:
        _body(ctx, tc, x.ap(), e_out.ap())
    nc.compile()
    return nc


def _body(ctx, tc, x, e_out):
    nc = tc.nc

    xpool = ctx.enter_context(tc.tile_pool(name="xin", bufs=1))
    spool = ctx.enter_context(tc.tile_pool(name="small", bufs=1))

    # ---- Stage 1: channel-sum reduction ----
    # Three DMA streams (two HWDGE rings + ONE SWDGE chunk: a single
    # gpsimd dma_start as the Pool engine's first instruction never blocks
    # its fold stream, and its data reaches gp folds without the ~1.6us DMA
    # sem hop). gpsimd folds chunk halves (once or twice) or fully reduces
    # pow2 tail chunks; DVE reduces the rest, in data-ready order.
    X = mybir.AxisListType.X
    K = len(PLAN)
    parts = spool.tile([P, K, HW], F32)
    engines = {0: nc.sync, 1: nc.scalar, 2: nc.gpsimd}

    xts = []
    coff = 0
    for k, (cc, q, m) in enumerate(PLAN):
        xt = xpool.tile([P, cc * HW], F32, tag=f"xt{k}")
        engines[q].dma_start(out=xt, in_=x[:, coff * HW:(coff + cc) * HW])
        xts.append(xt)
        coff += cc

    zs = {}
    for k in GP_ORDER:
        cc, q, m = PLAN[k]
        if m in ("f", "f2"):
            cur, w = xts[k], cc
            for lvl in range(1 if m == "f" else 2):
                half = (w // 2) * HW
                z = xpool.tile([P, half], F32, tag=f"z{k}_{lvl}")
                nc.gpsimd.tensor_add(z, cur[:, :half], cur[:, half:])
                cur, w = z, w // 2
            zs[k] = (cur, w)
        else:  # 'g': fold a pow2 chunk all the way down to one channel
            cur, w = xts[k], cc
            while w > 1:
                half = (w // 2) * HW
                if w // 2 == 1:
                    nc.gpsimd.tensor_add(parts[:, k, :],
                                         cur[:, :half], cur[:, half:])
                else:
                    z = xpool.tile([P, half], F32, tag=f"g{k}_{w}")
                    nc.gpsimd.tensor_add(z, cur[:, :half], cur[:, half:])
                    cur = z
                w //= 2

    def emit_reduce(k):
        cc, q, m = PLAN[k]
        src_t, w = zs[k] if m in ("f", "f2") else (xts[k], cc)
        nc.vector.reduce_sum(out=parts[:, k, :],
                             in_=src_t.rearrange("p (c h) -> p h c", c=w),
                             axis=X)

    for k in DVE_EARLY:
        emit_reduce(k)
    # early combine over the first K-2 columns (hidden behind tail reduces)
    xm_a = spool.tile([P, HW], F32)
    nc.vector.reduce_sum(
        out=xm_a, in_=parts[:, 0:K - 2, :].rearrange("p k h -> p h k"),
        axis=X)
    for k in DVE_LATE:
        emit_reduce(k)
    xm_b = spool.tile([P, HW], F32)
    nc.vector.tensor_add(xm_b, xm_a, parts[:, K - 2, :])
    xm_sum = spool.tile([P, HW], F32)
    nc.vector.tensor_add(xm_sum, xm_b, parts[:, K - 1, :])

    # ---- Stage 2: pairwise tail (tiny, all on-chip) ----
    # Partition layout is (b, n) b-major: each batch's 8 branch rows sit
    # within one 32-partition stream_shuffle quadrant, so rotating branches
    # by r gives xs[(b,i), :] = xm[(b,(i+r)%8), :] without any DMA.
    # sq is symmetric (bitwise: (a-b)^2 == (b-a)^2), so rotations 1..4 cover
    # all 28 unordered pairs (r=4 band twice); host mirrors the rest.
    R = N // 2  # rotations 1..4
    xs = spool.tile([P, R, HW], F32)
    for r in range(1, R + 1):
        mask = [(i & 24) | ((i + r) & 7) for i in range(32)]
        nc.vector.stream_shuffle(xs[:, r - 1, :], xm_sum, mask)

    # A[(b,i), (r,hw)] = xm[(b,i), hw] broadcast over r (stride-0 free dim)
    a_ap = xm_sum[:]
    A = bass.AP(tensor=a_ap.tensor, offset=a_ap.offset,
                ap=[list(a_ap.ap[0]), [0, R], [1, HW]])

    d = spool.tile([P, R, HW], F32)
    nc.vector.tensor_sub(d, A, xs)
    dsq = spool.tile([P, R, HW], F32)
    nc.vector.tensor_mul(dsq, d, d)

    sq_t = spool.tile([P, R], F32)
    nc.vector.reduce_sum(out=sq_t, in_=dsq, axis=mybir.AxisListType.X)

    e_t = spool.tile([P, R], F32)
    nc.scalar.activation(e_t, sq_t, mybir.ActivationFunctionType.Exp,
                         scale=EXP_SCALE)

    nc.sync.dma_start(out=e_out, in_=e_t)


_NC_CACHE = {}


def _get_nc():
    if "nc" not in _NC_CACHE:
        _NC_CACHE["nc"] = build_nc()
    return _NC_CACHE["nc"]


def _shard(x, c):
    """Core c's input: [b_local*8 + n, C*HW] (b-major partition order)."""
    xs = x[:, c * B_SH:(c + 1) * B_SH]              # [N, B_SH, C, H, W]
    return np.ascontiguousarray(
        xs.transpose(1, 0, 2, 3, 4).reshape(P, CF))


def kernel(x: np.ndarray):
    """Full-input entry point: x [8, 128, 512, 7, 7] f32 -> (direct, det, logdet)."""
    x = np.asarray(x, dtype=np.float32)
    assert x.shape == (N, B, C, 7, 7), x.shape
    nc = _get_nc()

    in_maps = [{"x": _shard(x, c)} for c in range(N_CORES)]
    res = run_bass_kernel_spmd(nc, in_maps, core_ids=list(range(N_CORES)))

    # e_out rows are (b_local, i); col r-1 holds exp(-g*sq(i, (i+r)%N, b))
    # for r=1..N/2. Assemble those bands, then mirror (sq is symmetric, so
    # the transposed entries are bitwise identical). Diagonal stays 0.
    R = N // 2
    rows = np.repeat(np.arange(N), R)
    cols = (np.arange(N)[:, None] + np.arange(1, R + 1)[None, :]).ravel() % N
    acc = np.zeros((N, N), dtype=np.float64)
    for c in range(N_CORES):
        e = res.results[c]["e_out"].astype(np.float64)  # [128, R]
        s = e.reshape(B_SH, N, R).sum(axis=0)           # [i, r-1]
        acc[rows, cols] += s.ravel()
    empty = acc == 0.0
    acc[empty] = acc.T[empty]
    snm = acc / B

    direct_div = snm.sum()
    det = np.linalg.det(snm)
    det_div = -det
    sign, logabs = np.linalg.slogdet(snm)
    logdet_div = -logabs if sign > 0 else np.float64(np.nan)

    return (
        np.float32(direct_div),
        np.float32(det_div),
        np.float32(logdet_div),
    )
